# revision 1
# baseline (speedup 1.0000x reference)
"""GraphTransformer (B=4, N=1024, H=8, D=256, L=4) on 8 TRN2 NeuronCores.

Sharding: core c -> (batch b = c//2, query-row half = c%2). Each core owns
512 query rows of one batch. k/v are computed for all 1024 rows locally
(from an AllGather'd hidden state); attention, residual+LN and FFN are
computed for the local 512 rows only. Between layers the updated local
rows are AllGather'd (bf16) within the 2-core pair that shares a batch.

Softmax uses exp(s + bias) = exp(s) * exp(bias) with exp(bias) precomputed
on host (bias values are tiny so no overflow; logits are O(+-4) so no
max-subtraction is needed in f32/bf16).
"""

import sys

sys.path.insert(0, "/opt/trn_rl_repo")

import numpy as np
import ml_dtypes

B, N, H, D, L = 4, 1024, 8, 256, 4
SVD = 16
DK = D // H  # 32
EPS = 1e-6
R = 512  # local query rows per core
NCORES = 8
SCALE = 1.0 / float(np.sqrt(DK))

_CACHE = {}


def _build_nc():
    import concourse.bass as bass
    from concourse import bacc, mybir, tile
    from concourse.masks import make_identity

    f32 = mybir.dt.float32
    bf16 = mybir.dt.bfloat16
    AF = mybir.ActivationFunctionType

    nc = bacc.Bacc(
        "TRN2",
        target_bir_lowering=False,
        debug=False,
        num_devices=NCORES,
    )

    # ---- kernel I/O ----
    xT = nc.dram_tensor("xT", [D, N], bf16, kind="ExternalInput").ap()
    xTloc = nc.dram_tensor("xTloc", [D, R], bf16, kind="ExternalInput").ap()
    xloc = nc.dram_tensor("xloc", [R, D], f32, kind="ExternalInput").ap()
    expbT = nc.dram_tensor("expbT", [H, N, R], bf16, kind="ExternalInput").ap()
    wts = {
        nm: nc.dram_tensor(nm, [L, 2, 128, D], bf16, kind="ExternalInput").ap()
        for nm in ["wq", "wk", "wv", "wa", "w1", "w2"]
    }
    out = nc.dram_tensor("out", [R, D], f32, kind="ExternalOutput").ap()

    groups = [[0, 1], [2, 3], [4, 5], [6, 7]]

    with tile.TileContext(nc) as tc:
        with (
            tc.tile_pool(name="const", bufs=1) as const,
            tc.tile_pool(name="state", bufs=2) as state,
            tc.tile_pool(name="work", bufs=3) as work,
            tc.tile_pool(name="ps", bufs=4, space="PSUM") as ps_pool,
            tc.tile_pool(name="ops", bufs=2, space="PSUM") as ops_pool,
            tc.tile_pool(name="dram", bufs=2, space="DRAM") as dram,
        ):
            # ---- constants ----
            idbf = const.tile([128, 128], bf16, tag="idbf")
            make_identity(nc, idbf)
            idf = const.tile([128, 128], f32, tag="idf")
            make_identity(nc, idf)
            eps_t = const.tile([128, 1], f32, tag="eps")
            nc.vector.memset(eps_t, EPS)
            ones_r = const.tile([1, 32], f32, tag="ones_r")
            nc.vector.memset(ones_r, 1.0)

            # weights resident: [128, L, 2, D] per matrix
            w_sb = {}
            for nm in wts:
                t = const.tile([128, L, 2, D], bf16, tag=f"w_{nm}")
                src = wts[nm]  # [L, 2, 128, D]
                nc.sync.dma_start(
                    out=t, in_=src.rearrange("l c p d -> p l c d")
                )
                w_sb[nm] = t

            # ---- initial state ----
            # hT: transposed full-row hidden [2][128, N] bf16
            hT = []
            for dt_ in range(2):
                t = state.tile([128, N], bf16, tag=f"hT{dt_}")
                nc.sync.dma_start(out=t, in_=xT[dt_ * 128 : (dt_ + 1) * 128, :])
                hT.append(t)
            # hTloc: transposed local-row hidden [2][128, R] bf16
            hTloc = []
            for dt_ in range(2):
                t = state.tile([128, R], bf16, tag=f"hTl{dt_}")
                nc.sync.dma_start(out=t, in_=xTloc[dt_ * 128 : (dt_ + 1) * 128, :])
                hTloc.append(t)
            # h_loc: natural local rows [4][128, D] f32
            h_loc = []
            for it in range(4):
                t = state.tile([128, D], f32, tag=f"h{it}")
                nc.sync.dma_start(out=t, in_=xloc[it * 128 : (it + 1) * 128, :])
                h_loc.append(t)

            # exp(bias) resident: 8 tiles [128(j), H, R] bf16
            expb_sb = []
            for jc in range(8):
                t = const.tile([128, H, R], bf16, tag=f"expb{jc}")
                nc.sync.dma_start(
                    out=t,
                    in_=expbT[:, jc * 128 : (jc + 1) * 128, :].rearrange(
                        "h j i -> j h i"
                    ),
                )
                expb_sb.append(t)


            def layer_norm(z_ps, resid_sb, out_sb):
                """out_sb = LN(resid_sb + z_ps), gains=1 biases=0."""
                zs = work.tile([128, D], f32, tag="ln_z")
                nc.vector.tensor_add(zs, z_ps, resid_sb)
                stats = work.tile([128, 6], f32, tag="ln_stats")
                nc.vector.bn_stats(out=stats, in_=zs)
                mv = work.tile([128, 2], f32, tag="ln_mv")
                nc.vector.bn_aggr(out=mv, in_=stats)
                std = work.tile([128, 1], f32, tag="ln_std")
                nc.scalar.activation(
                    out=std, in_=mv[:, 1:2], func=AF.Sqrt, bias=eps_t
                )
                rstd = work.tile([128, 1], f32, tag="ln_rstd")
                nc.vector.reciprocal(rstd, std)
                nc.vector.tensor_scalar(
                    out=out_sb,
                    in0=zs,
                    scalar1=mv[:, 0:1],
                    scalar2=rstd,
                    op0=mybir.AluOpType.subtract,
                    op1=mybir.AluOpType.mult,
                )
                return zs

            for t in range(L):
                # ---- qT [2][128, R] bf16 (scaled by SCALE) ----
                qT = []
                for od in range(2):
                    pst = ps_pool.tile([128, R], f32, tag="gen")
                    for dt_ in range(2):
                        nc.tensor.matmul(
                            pst,
                            w_sb["wq"][:, t, dt_, od * 128 : (od + 1) * 128],
                            hTloc[dt_],
                            start=(dt_ == 0),
                            stop=(dt_ == 1),
                        )
                    sb = work.tile([128, R], bf16, tag=f"qT{od}", bufs=2)
                    nc.vector.tensor_copy(out=sb, in_=pst)
                    qT.append(sb)

                # ---- kT [2][128, N] bf16 ----
                kT = []
                for od in range(2):
                    sb = work.tile([128, N], bf16, tag=f"kT{od}", bufs=2)
                    for jh in range(2):
                        pst = ps_pool.tile([128, 512], f32, tag="gen")
                        for dt_ in range(2):
                            nc.tensor.matmul(
                                pst,
                                w_sb["wk"][:, t, dt_, od * 128 : (od + 1) * 128],
                                hT[dt_][:, jh * 512 : (jh + 1) * 512],
                                start=(dt_ == 0),
                                stop=(dt_ == 1),
                            )
                        nc.vector.tensor_copy(
                            out=sb[:, jh * 512 : (jh + 1) * 512], in_=pst
                        )
                    kT.append(sb)

                # PE matmul base partition must be 0/32/64 — heads 3 and 7
                # sit at offset 96, so copy them down to offset 0.
                fix = {}
                for od in range(2):
                    kfx = work.tile([128, N], bf16, tag=f"kfx{od}", name=f"kfx{od}", bufs=1)
                    nc.vector.tensor_copy(out=kfx[0:32, :], in_=kT[od][96:128, :])
                    qfx = work.tile([128, R], bf16, tag=f"qfx{od}", name=f"qfx{od}", bufs=1)
                    nc.vector.tensor_copy(out=qfx[0:32, :], in_=qT[od][96:128, :])
                    fix[od] = (kfx, qfx)

                def kq_aps(h):
                    od, pr = h // 4, (h % 4) * 32
                    if pr == 96:
                        kfx, qfx = fix[od]
                        return kfx, qfx, 0
                    return kT[od], qT[od], pr

                # ---- v_aug [8][128, H, 33] bf16 (col 32 = ones) ----
                v_aug = []
                for rt in range(8):
                    sb = work.tile([128, H, 33], bf16, tag=f"v{rt}", bufs=1)
                    nc.vector.memset(sb[:, :, 32:33], 1.0)
                    pst = ps_pool.tile([128, D], f32, tag="gen")
                    for dt_ in range(2):
                        nc.tensor.matmul(
                            pst,
                            hT[dt_][:, rt * 128 : (rt + 1) * 128],
                            w_sb["wv"][:, t, dt_, :],
                            start=(dt_ == 0),
                            stop=(dt_ == 1),
                        )
                    nc.vector.tensor_copy(
                        out=sb[:, :, 0:32],
                        in_=pst.rearrange("p (h d) -> p h d", h=H),
                    )
                    v_aug.append(sb)

                # ---- attention ----
                o_ps = [ops_pool.tile([128, H, 33], f32, tag="o_ps", name=f"o_ps{_i}", bufs=4) for _i in range(4)]
                for h in range(H):
                    k_src, q_src, pr = kq_aps(h)
                    e_t = work.tile([128, 8, R], bf16, tag="e", bufs=2)
                    for jc in range(8):
                        st = ps_pool.tile([128, R], f32, tag="gen")
                        nc.tensor.matmul(
                            st,
                            k_src[pr : pr + 32, jc * 128 : (jc + 1) * 128],
                            q_src[pr : pr + 32, :],
                            start=True,
                            stop=True,
                        )
                        etmp = work.tile([128, R], bf16, tag="etmp")
                        nc.scalar.activation(out=etmp, in_=st, func=AF.Exp)
                        nc.vector.tensor_mul(
                            e_t[:, jc, :], etmp, expb_sb[jc][:, h, :]
                        )
                    for it in range(4):
                        for jc in range(8):
                            nc.tensor.matmul(
                                o_ps[it][:, h, :],
                                e_t[:, jc, it * 128 : (it + 1) * 128],
                                v_aug[jc][:, h, :],
                                start=(jc == 0),
                                stop=(jc == 7),
                            )

                # ---- normalize o, transpose -> oT ----
                oT = [work.tile([128, R], bf16, tag=f"oT{dh}", name=f"oT{dh}", bufs=2) for dh in range(2)]
                for it in range(4):
                    den = work.tile([128, H], f32, tag="den")
                    nc.vector.reciprocal(den, o_ps[it][:, :, 32])
                    o_sb = work.tile([128, D], bf16, tag="o_sb", bufs=2)
                    for h in range(H):
                        nc.vector.tensor_scalar(
                            out=o_sb[:, h * 32 : (h + 1) * 32],
                            in0=o_ps[it][:, h, 0:32],
                            scalar1=den[:, h : h + 1],
                            scalar2=None,
                            op0=mybir.AluOpType.mult,
                        )
                    for dh in range(2):
                        tp = ps_pool.tile([128, 128], bf16, tag="gen", name="tp")
                        nc.tensor.transpose(
                            tp, o_sb[:, dh * 128 : (dh + 1) * 128], idbf
                        )
                        nc.vector.tensor_copy(
                            out=oT[dh][:, it * 128 : (it + 1) * 128], in_=tp
                        )

                # ---- attn out + residual + LN -> h1 ----
                h1 = []
                for it in range(4):
                    pst = ps_pool.tile([128, D], f32, tag="gen")
                    for dt_ in range(2):
                        nc.tensor.matmul(
                            pst,
                            oT[dt_][:, it * 128 : (it + 1) * 128],
                            w_sb["wa"][:, t, dt_, :],
                            start=(dt_ == 0),
                            stop=(dt_ == 1),
                        )
                    h1t = work.tile([128, D], f32, tag=f"h1_{it}", bufs=2)
                    layer_norm(pst, h_loc[it], h1t)
                    h1.append(h1t)

                # ---- h1T bf16 ----
                h1T = [work.tile([128, R], bf16, tag=f"h1T{dh}", name=f"h1T{dh}", bufs=2) for dh in range(2)]
                for it in range(4):
                    for dh in range(2):
                        tp = ps_pool.tile([128, 128], f32, tag="gen")
                        nc.tensor.transpose(
                            tp, h1[it][:, dh * 128 : (dh + 1) * 128], idf
                        )
                        nc.vector.tensor_copy(
                            out=h1T[dh][:, it * 128 : (it + 1) * 128], in_=tp
                        )

                # ---- FFN: f1T = relu(W1^T h1T), f2 = f1 @ W2 ----
                f1T = []
                for od in range(2):
                    pst = ps_pool.tile([128, R], f32, tag="gen")
                    for dt_ in range(2):
                        nc.tensor.matmul(
                            pst,
                            w_sb["w1"][:, t, dt_, od * 128 : (od + 1) * 128],
                            h1T[dt_],
                            start=(dt_ == 0),
                            stop=(dt_ == 1),
                        )
                    sb = work.tile([128, R], bf16, tag=f"f1T{od}", bufs=2)
                    nc.vector.tensor_scalar_max(sb, pst, 0.0)
                    f1T.append(sb)

                h2 = []
                h2_bf = []
                for it in range(4):
                    pst = ps_pool.tile([128, D], f32, tag="gen")
                    for dh in range(2):
                        nc.tensor.matmul(
                            pst,
                            f1T[dh][:, it * 128 : (it + 1) * 128],
                            w_sb["w2"][:, t, dh, :],
                            start=(dh == 0),
                            stop=(dh == 1),
                        )
                    h2t = state.tile([128, D], f32, tag=f"h{it}")
                    layer_norm(pst, h1[it], h2t)
                    h2.append(h2t)
                    if t < L - 1:
                        hb = work.tile([128, D], bf16, tag=f"h2b{it}", bufs=2)
                        nc.vector.tensor_copy(out=hb, in_=h2t)
                        h2_bf.append(hb)

                h_loc = h2

                if t < L - 1:
                    # ---- allgather local rows within pair, rebuild hT ----
                    cc_in = dram.tile([R, D], bf16, tag="cc_in")
                    cc_out = dram.tile([N, D], bf16, tag="cc_out")
                    for it in range(4):
                        nc.sync.dma_start(
                            out=cc_in[it * 128 : (it + 1) * 128, :], in_=h2_bf[it]
                        )
                    nc.gpsimd.collective_compute(
                        "AllGather",
                        mybir.AluOpType.bypass,
                        replica_groups=groups,
                        ins=[cc_in.opt()],
                        outs=[cc_out.opt()],
                    )
                    hT = []
                    for dt_ in range(2):
                        nt = state.tile([128, N], bf16, tag=f"hT{dt_}")
                        nc.sync.dma_start_transpose(
                            out=nt, in_=cc_out[:, dt_ * 128 : (dt_ + 1) * 128]
                        )
                        hT.append(nt)
                    hTloc = []
                    for dt_ in range(2):
                        nt = state.tile([128, R], bf16, tag=f"hTl{dt_}")
                        nc.sync.dma_start_transpose(
                            out=nt, in_=cc_in[:, dt_ * 128 : (dt_ + 1) * 128]
                        )
                        hTloc.append(nt)

            for it in range(4):
                nc.sync.dma_start(
                    out=out[it * 128 : (it + 1) * 128, :], in_=h_loc[it]
                )

    nc.compile()
    return nc


def _get_nc():
    if "nc" not in _CACHE:
        _CACHE["nc"] = _build_nc()
    return _CACHE["nc"]


def _host_prep(inputs):
    bf = ml_dtypes.bfloat16
    x = np.asarray(inputs["x"], np.float32)
    in_deg = np.asarray(inputs["in_degrees"]).astype(np.int64)
    out_deg = np.asarray(inputs["out_degrees"]).astype(np.int64)
    sp = np.asarray(inputs["spatial_pos"]).astype(np.int64)
    svd = np.asarray(inputs["svd_emb"], np.float32)

    pre = (
        np.asarray(inputs["in_deg_emb"], np.float32)[in_deg]
        + np.asarray(inputs["out_deg_emb"], np.float32)[out_deg]
    )
    pos = np.concatenate([svd[:, :SVD], -svd[:, SVD:]], axis=-1)
    pre = pre + pos @ np.asarray(inputs["W_svd"], np.float32) + np.asarray(
        inputs["b_svd"], np.float32
    )
    xp = x + pre[None]  # [B, N, D]

    expb = np.exp(np.asarray(inputs["spatial_emb"], np.float32)[sp])  # [N, N, H]

    w_payload = {}
    for key, nm in [
        ("Wq", "wq"),
        ("Wk", "wk"),
        ("Wv", "wv"),
        ("Wa", "wa"),
        ("W1", "w1"),
        ("W2", "w2"),
    ]:
        w = np.asarray(inputs[key], np.float32)  # [L, D, D]
        if nm == "wq":
            w = w * SCALE
        w_payload[nm] = np.ascontiguousarray(
            w.reshape(L, 2, 128, D).astype(bf)
        )

    in_maps = []
    for c in range(NCORES):
        b, half = c // 2, c % 2
        r0 = half * R
        xb = xp[b]  # [N, D] f32
        m = {
            "xT": np.ascontiguousarray(xb.T.astype(bf)),
            "xTloc": np.ascontiguousarray(xb[r0 : r0 + R].T.astype(bf)),
            "xloc": np.ascontiguousarray(xb[r0 : r0 + R]),
            # expbT[h, j, i] = expb[r0+i, j, h]
            "expbT": np.ascontiguousarray(
                expb[r0 : r0 + R].transpose(2, 1, 0).astype(bf)
            ),
        }
        m.update(w_payload)
        in_maps.append(m)
    return in_maps


def kernel(**inputs):
    from concourse.bass_utils import run_bass_kernel_spmd

    nc = _get_nc()
    in_maps = _host_prep(inputs)
    res = run_bass_kernel_spmd(nc, in_maps, core_ids=list(range(NCORES)))
    out = np.empty((B, N, D), np.float32)
    for c in range(NCORES):
        b, half = c // 2, c % 2
        out[b, half * R : (half + 1) * R] = res.results[c]["out"]
    return out


if __name__ == "__main__":
    nc = _get_nc()
    print("compiled OK")

